# revision 6
# baseline (speedup 1.0000x reference)
import sys
from contextlib import ExitStack

import numpy as np

sys.path.insert(0, "/opt/trn_rl_repo")

import concourse.bass as bass  # noqa: E402
import concourse.mybir as mybir  # noqa: E402
import concourse.tile as tile  # noqa: E402
from concourse import bacc  # noqa: E402
from concourse.bass_utils import run_bass_kernel_spmd  # noqa: E402

C = 64
N_CORES = 8

# Pair table: each entry computes taps (ka, kb) of the 3x3 dynamic filter in
# one [K=64, M=128] matmul (top 64 psum partitions = ka's channels, bottom 64
# = kb's channels).  `tl` selects which staged plane-stack supplies the
# shifted patch operand; `r` is the row offset of the window within the
# padded 128-wide row layout.  Tap k=5 appears in both B-pairs with halved
# weights/bias so the products sum to the correct single contribution.
#   XA stack: top 64 partitions = padded x at col-offset 0 (j=0), bottom 64
#             = col-offset 1 (j=1).  Pair (3e, 3e+1) reads rows h0+e.
#   XB stack: top = col-offset 2 (j=2), bottom = j=2 shifted down one row.
PAIRS = [
    (0, 1, "A", 0),
    (3, 4, "A", 1),
    (6, 7, "A", 2),
    (2, 5, "B", 0),
    (5, 8, "B", 1),
]

F16 = mybir.dt.float16
F32 = mybir.dt.float32


def pack_weights(w_gen: np.ndarray, b_gen: np.ndarray):
    """Host-side packing of the tiny 1x1 generator weights.

    Returns
      wt   [128, 5*128] fp16 : lhsT blocks; block j col (half*64+c) holds
                               W[c, k, :] (tap k of pair j, half-scaled for
                               k=5); rows duplicated so PE row-group B
                               (partitions 64-127) can read its stationary
                               from the matching partition range
      bias [128, 5]    fp32 : per-partition bias for pair j
      idt  [128, 64]   fp16 : stacked identity [I64; I64] for the fold matmul
    """
    W = w_gen.reshape(C, 9, C).astype(np.float32)  # [c, k, c']
    b = b_gen.reshape(C, 9).astype(np.float32)
    wt = np.zeros((C, 5 * 128), np.float32)
    bias = np.zeros((128, 5), np.float32)
    for jdx, (ka, kb, _, _) in enumerate(PAIRS):
        for half, k in ((0, ka), (1, kb)):
            s = 0.5 if k == 5 else 1.0
            wt[:, jdx * 128 + half * 64 : jdx * 128 + half * 64 + C] = W[:, k, :].T * s
            bias[half * 64 : half * 64 + C, jdx] = b[:, k] * s
    idt = np.concatenate([np.eye(C), np.eye(C)], axis=0)
    wt2 = np.concatenate([wt, wt], axis=0)
    return wt2.astype(np.float16), bias.astype(np.float32), idt.astype(np.float16)


def stage_planes(x_core: np.ndarray, H: int, W: int):
    """Host-side staging: three 128-wide-row fp16 plane stacks whose window
    reads are all contiguous in the free dimension.

    With P = zero-padded x ([C, H+2, W+2]) and J_j[c, a, u] = P[c, a, u+j]:
      XA [128, (H+2)*W]: top 64 = J0, bottom 64 = J1
      XB [128, (H+2)*W]: top 64 = J2, bottom 64 = J2 shifted down one row
      XC [64,  (H+2)*W]: J1 (f-matmul rhs for PE row group A)
    """
    P = np.zeros((C, H + 2, W + 2), np.float16)
    P[:, 1 : H + 1, 1 : W + 1] = x_core
    J0 = P[:, :, 0:W].reshape(C, -1)
    J1 = P[:, :, 1 : W + 1].reshape(C, -1)
    J2 = P[:, :, 2 : W + 2].reshape(C, -1)
    J2dn = np.zeros_like(J2)
    J2dn[:, : -W] = J2[:, W:]  # row a -> row a+1
    XA = np.concatenate([J0, J1], axis=0)
    XB = np.concatenate([J2, J2dn], axis=0)
    return (
        np.ascontiguousarray(XA),
        np.ascontiguousarray(XB),
        np.ascontiguousarray(J1),
    )


def build_nc(H=128, W=128, CH=4):
    """Build the single-core Bass program (SPMD across cores).

    H, W: spatial dims; CH: image rows per chunk.
    """
    PS = (H + 2) * W  # plane size (rows of width W)
    Nc = CH * W  # pixels per chunk
    nch = H // CH
    assert Nc <= 512  # one psum bank / one matmul

    nc = bacc.Bacc("TRN2", target_bir_lowering=False)
    xa_in = nc.declare_dram_parameter("xa", [128, PS], F16, isOutput=False)
    xb_in = nc.declare_dram_parameter("xb", [128, PS], F16, isOutput=False)
    xc_in = nc.declare_dram_parameter("xc", [C, PS], F16, isOutput=False)
    wt_in = nc.declare_dram_parameter("wt", [128, 5 * 128], F16, isOutput=False)
    bias_in = nc.declare_dram_parameter("bias", [128, 5], F32, isOutput=False)
    idt_in = nc.declare_dram_parameter("idt", [128, C], F16, isOutput=False)
    out_ext = nc.declare_dram_parameter("out", [C, H, W], F16, isOutput=True)

    add = mybir.AluOpType.add
    mult = mybir.AluOpType.mult
    Identity = mybir.ActivationFunctionType.Identity

    N_EXTRACT = 3  # f-tiles extracted by ScalarE (bias fused there)
    GROUP_B = (3, 4)  # pairs streaming through PE rows 64-127
    EMIT_ORDER = [0, 3, 1, 4, 2]  # alternate A/B row-groups on the PE

    with ExitStack() as ctx:
        tc = ctx.enter_context(tile.TileContext(nc))
        const = ctx.enter_context(tc.tile_pool(name="const", bufs=1))
        fpsum = ctx.enter_context(tc.tile_pool(name="fpsum", bufs=6, space="PSUM"))
        opsum = ctx.enter_context(tc.tile_pool(name="opsum", bufs=2, space="PSUM"))
        fbp = ctx.enter_context(tc.tile_pool(name="fbp", bufs=3))
        ptp = ctx.enter_context(tc.tile_pool(name="ptp", bufs=4))
        pp = ctx.enter_context(tc.tile_pool(name="pp", bufs=6))
        outp = ctx.enter_context(tc.tile_pool(name="outp", bufs=4))

        XA = const.tile([128, PS], F16)
        XB = const.tile([128, PS], F16)
        XC = const.tile([C, PS], F16)
        WT = const.tile([128, 5 * 128], F16)
        BIAS = const.tile([128, 5], F32)
        IDT = const.tile([128, C], F16)

        # banded loads of the host-staged planes; the first chunk's needs
        # issue first so compute starts early.  WT/BIAS/IDT and the XC tail
        # go on the (otherwise idle) GPSIMD SWDGE queue so they load in
        # parallel with the Sync engine's band loads.
        # Three independent DMA queues carry the three staged planes so the
        # early bands land in parallel: XA on Sync (HWDGE), XB on the GPSIMD
        # SWDGE queue, XC on Scalar (its issue ops run before ScalarE's
        # first extract).  WT leads the Sync queue; BIAS/IDT lead GPSIMD's.
        bands = [0, 6, 24, 48, 80, H + 2]
        nc.sync.dma_start(WT[:], wt_in[:])
        nc.gpsimd.dma_start(BIAS[:], bias_in[:])
        nc.gpsimd.dma_start(IDT[:], idt_in[:])
        for b in range(len(bands) - 1):
            r0, r1 = bands[b], bands[b + 1]
            nc.scalar.dma_start(XC[:, r0 * W : r1 * W], xc_in[:, r0 * W : r1 * W])
            nc.sync.dma_start(XA[:, r0 * W : r1 * W], xa_in[:, r0 * W : r1 * W])
            nc.gpsimd.dma_start(XB[:, r0 * W : r1 * W], xb_in[:, r0 * W : r1 * W])

        def trio_window(off, count):
            """[128, count, Nc] sliding window over XA: consecutive windows
            start one row (W elems) apart; each spans CH rows contiguously."""
            base = XA[:, off : off + 1]
            w = base.copy()
            w.ap = mybir.VecI64Pair([tuple(w.ap[0]), (W, count), (1, Nc)])
            return w

        def fold(op_tile, n, src_ap, start, stop):
            """Fold matmul: contract the pair halves of one product tile into
            the op psum; even chunks land on psum partitions 0-63 (PE col
            group 0), odd chunks on 64-127 (col group 64)."""
            half = n % 2
            nc.tensor.matmul(
                op_tile[half * C : half * C + C, :],
                IDT[:],
                src_ap,
                start=start,
                stop=stop,
                tile_position=(0, half * C),
            )

        def evacuate(op_tile, g0, final=False):
            OUT = outp.tile([128, Nc], F16)
            nc.scalar.copy(OUT[:], op_tile[:])
            o3 = OUT[:].rearrange("p (a b) -> p a b", a=CH)
            # the last evacuation issues its stores on two queues in parallel
            eng = nc.scalar if final else nc.sync
            eng.dma_start(out_ext[:, g0 : g0 + CH, :], o3[0:C])
            nc.sync.dma_start(out_ext[:, g0 + CH : g0 + 2 * CH, :], o3[C:128])

        def emit_folds(op_tile, cn, srcs):
            for fi, src in enumerate(srcs):
                fold(
                    op_tile, cn, src,
                    start=(fi == 0), stop=(fi == len(srcs) - 1),
                )

        op = None  # op psum tile; covers chunks (2m, 2m+1)
        prev_folds = None  # chunk n-1's five product tiles (folded in chunk n)

        for n in range(nch):
            h0 = n * CH
            Ps = {}
            FB = fbp.tile([128, N_EXTRACT * Nc], F16, tag="fb")
            PT = ptp.tile([128, N_EXTRACT * Nc], F16, tag="pt")
            for jdx in EMIT_ORDER:
                ka, kb, tl, r = PAIRS[jdx]
                fp = fpsum.tile([128, Nc], F32, tag="fp")
                grp_b = jdx in GROUP_B
                r0 = h0 + 1
                if grp_b:
                    lhsT = WT[C:128, jdx * 128 : (jdx + 1) * 128]
                    rhs = XA[C:128, r0 * W : r0 * W + Nc]
                    tpos = (64, 0)
                else:
                    lhsT = WT[0:C, jdx * 128 : (jdx + 1) * 128]
                    rhs = XC[0:C, r0 * W : r0 * W + Nc]
                    tpos = (0, 0)
                nc.tensor.matmul(
                    fp[:], lhsT, rhs, start=True, stop=True, tile_position=tpos
                )
                if jdx < N_EXTRACT:
                    # ScalarE evacuates f (+bias) to SBUF fp16
                    nc.scalar.activation(
                        FB[:, jdx * Nc : (jdx + 1) * Nc],
                        fp[:],
                        Identity,
                        bias=BIAS[:, jdx : jdx + 1],
                    )
                else:
                    # DVE reads f straight from PSUM (1x) with bias fused
                    src = XA if tl == "A" else XB
                    in1 = src[:, (h0 + r) * W : (h0 + r) * W + Nc]
                    P = pp.tile([128, Nc], F16, tag="p")
                    nc.vector.scalar_tensor_tensor(
                        P[:], fp[:], BIAS[:, jdx : jdx + 1], in1, add, mult
                    )
                    Ps[jdx] = P[:]
            # one DVE op computes all three ScalarE-extracted pair products;
            # the last chunk uses three single-window ops instead so each
            # epilogue fold can start as soon as its own product is done
            FB3 = FB[:].rearrange("p (e n) -> p e n", e=N_EXTRACT)
            PT3 = PT[:].rearrange("p (e n) -> p e n", e=N_EXTRACT)
            if n < nch - 1:
                nc.vector.tensor_tensor(
                    PT3, FB3, trio_window(h0 * W, N_EXTRACT), mult
                )
            else:
                for e in range(N_EXTRACT):
                    nc.vector.tensor_tensor(
                        PT3[:, e : e + 1],
                        FB3[:, e : e + 1],
                        trio_window((h0 + e) * W, 1),
                        mult,
                    )

            # all of chunk n-1's folds run here: every product is a full
            # chunk old, so the PE never stalls on the DVE mid-chunk
            if prev_folds is not None:
                if n % 2 == 1:
                    op = opsum.tile([128, Nc], F32, tag="op")
                emit_folds(op, n - 1, prev_folds)
                if n % 2 == 0:
                    evacuate(op, (n - 2) * CH)
            prev_folds = [Ps[3], Ps[4]] + [
                PT[:, e * Nc : (e + 1) * Nc] for e in range(N_EXTRACT)
            ]

        # epilogue: last chunk's folds + final op evacuation
        emit_folds(op, nch - 1, prev_folds)
        evacuate(op, (nch - 2) * CH, final=True)

    nc.compile()
    return nc


_NC_CACHE = {}


def _get_nc(H, W, CH):
    key = (H, W, CH)
    if key not in _NC_CACHE:
        _NC_CACHE[key] = build_nc(H, W, CH)
    return _NC_CACHE[key]


def run(x, w_gen, b_gen, trace=False, tmpdir=None):
    x = np.asarray(x, dtype=np.float32)
    w_gen = np.asarray(w_gen, dtype=np.float32)
    b_gen = np.asarray(b_gen, dtype=np.float32)
    B, c, H, W = x.shape
    assert c == C and B == N_CORES

    wt, bias, idt = pack_weights(w_gen, b_gen)
    nc = _get_nc(H, W, 4)

    in_maps = []
    for i in range(B):
        XA, XB, XC = stage_planes(x[i], H, W)
        in_maps.append(
            {"xa": XA, "xb": XB, "xc": XC, "wt": wt, "bias": bias, "idt": idt}
        )
    res = run_bass_kernel_spmd(
        nc, in_maps, core_ids=list(range(N_CORES)), trace=trace, tmpdir=tmpdir
    )
    out = np.stack(
        [res.results[i]["out"].astype(np.float32) for i in range(B)], axis=0
    )
    return out, res


def kernel(x: np.ndarray, w_gen: np.ndarray, b_gen: np.ndarray) -> np.ndarray:
    return run(x, w_gen, b_gen)[0]


# revision 8
# speedup vs baseline: 1.0446x; 1.0446x over previous
import sys
from contextlib import ExitStack

import numpy as np

sys.path.insert(0, "/opt/trn_rl_repo")

import concourse.bass as bass  # noqa: E402
import concourse.mybir as mybir  # noqa: E402
import concourse.tile as tile  # noqa: E402
from concourse import bacc  # noqa: E402
from concourse.bass_utils import run_bass_kernel_spmd  # noqa: E402

C = 64
N_CORES = 8

# Pair table: each entry computes taps (ka, kb) of the 3x3 dynamic filter in
# one [K=64, M=128] matmul (top 64 psum partitions = ka's channels, bottom 64
# = kb's channels).  `tl` selects which staged plane-stack supplies the
# shifted patch operand; `r` is the row offset of the window within the
# padded 128-wide row layout.  Tap k=5 appears in both B-pairs with halved
# weights/bias so the products sum to the correct single contribution.
#   XA stack: top 64 partitions = padded x at col-offset 0 (j=0), bottom 64
#             = col-offset 1 (j=1).  Pair (3e, 3e+1) reads rows h0+e.
#   XB stack: top = col-offset 2 (j=2), bottom = j=2 shifted down one row.
PAIRS = [
    (0, 1, "A", 0),
    (3, 4, "A", 1),
    (6, 7, "A", 2),
    (2, 5, "B", 0),
    (5, 8, "B", 1),
]

F16 = mybir.dt.float16
F32 = mybir.dt.float32


def pack_weights(w_gen: np.ndarray, b_gen: np.ndarray):
    """Host-side packing of the tiny 1x1 generator weights.

    Returns
      wt   [128, 5*128] fp16 : lhsT blocks; block j col (half*64+c) holds
                               W[c, k, :] (tap k of pair j, half-scaled for
                               k=5); rows duplicated so PE row-group B
                               (partitions 64-127) can read its stationary
                               from the matching partition range
      bias [128, 5]    fp32 : per-partition bias for pair j
      idt  [128, 64]   fp16 : stacked identity [I64; I64] for the fold matmul
    """
    W = w_gen.reshape(C, 9, C).astype(np.float32)  # [c, k, c']
    b = b_gen.reshape(C, 9).astype(np.float32)
    wt = np.zeros((C, 5 * 128), np.float32)
    bias = np.zeros((128, 5), np.float32)
    for jdx, (ka, kb, _, _) in enumerate(PAIRS):
        for half, k in ((0, ka), (1, kb)):
            s = 0.5 if k == 5 else 1.0
            wt[:, jdx * 128 + half * 64 : jdx * 128 + half * 64 + C] = W[:, k, :].T * s
            bias[half * 64 : half * 64 + C, jdx] = b[:, k] * s
    idt = np.concatenate([np.eye(C), np.eye(C)], axis=0)
    wt2 = np.concatenate([wt, wt], axis=0)
    return wt2.astype(np.float16), bias.astype(np.float32), idt.astype(np.float16)


def stage_planes(x_core: np.ndarray, H: int, W: int):
    """Host-side staging: three 128-wide-row fp16 plane stacks whose window
    reads are all contiguous in the free dimension.

    With P = zero-padded x ([C, H+2, W+2]) and J_j[c, a, u] = P[c, a, u+j]:
      XA [128, (H+2)*W]: top 64 = J0, bottom 64 = J1
      XB [128, (H+2)*W]: top 64 = J2, bottom 64 = J2 shifted down one row
      XC [64,  (H+2)*W]: J1 (f-matmul rhs for PE row group A)
    """
    P = np.zeros((C, H + 2, W + 2), np.float16)
    P[:, 1 : H + 1, 1 : W + 1] = x_core
    J0 = P[:, :, 0:W].reshape(C, -1)
    J1 = P[:, :, 1 : W + 1].reshape(C, -1)
    J2 = P[:, :, 2 : W + 2].reshape(C, -1)
    J2dn = np.zeros_like(J2)
    J2dn[:, : -W] = J2[:, W:]  # row a -> row a+1
    XA = np.concatenate([J0, J1], axis=0)
    XB = np.concatenate([J2, J2dn], axis=0)
    return (
        np.ascontiguousarray(XA),
        np.ascontiguousarray(XB),
        np.ascontiguousarray(J1),
    )


def build_nc(H=128, W=128, CH=4):
    """Build the single-core Bass program (SPMD across cores).

    H, W: spatial dims; CH: image rows per chunk.
    """
    PS = (H + 2) * W  # plane size (rows of width W)
    Nc = CH * W  # pixels per chunk
    nch = H // CH
    assert Nc <= 512  # one psum bank / one matmul

    nc = bacc.Bacc("TRN2", target_bir_lowering=False)
    xa_in = nc.declare_dram_parameter("xa", [128, PS], F16, isOutput=False)
    xb_in = nc.declare_dram_parameter("xb", [128, PS], F16, isOutput=False)
    xc_in = nc.declare_dram_parameter("xc", [C, PS], F16, isOutput=False)
    wt_in = nc.declare_dram_parameter("wt", [128, 5 * 128], F16, isOutput=False)
    bias_in = nc.declare_dram_parameter("bias", [128, 5], F32, isOutput=False)
    idt_in = nc.declare_dram_parameter("idt", [128, C], F16, isOutput=False)
    out_ext = nc.declare_dram_parameter("out", [C, H, W], F16, isOutput=True)

    add = mybir.AluOpType.add
    mult = mybir.AluOpType.mult
    Identity = mybir.ActivationFunctionType.Identity

    N_EXTRACT = 3  # f-tiles extracted by ScalarE (bias fused there)
    GROUP_B = (3, 4)  # pairs streaming through PE rows 64-127
    EMIT_ORDER = [0, 3, 1, 4, 2]  # alternate A/B row-groups on the PE

    with ExitStack() as ctx:
        tc = ctx.enter_context(tile.TileContext(nc))
        const = ctx.enter_context(tc.tile_pool(name="const", bufs=1))
        fpsum = ctx.enter_context(tc.tile_pool(name="fpsum", bufs=6, space="PSUM"))
        opsum = ctx.enter_context(tc.tile_pool(name="opsum", bufs=2, space="PSUM"))
        fbp = ctx.enter_context(tc.tile_pool(name="fbp", bufs=3))
        ptp = ctx.enter_context(tc.tile_pool(name="ptp", bufs=4))
        pp = ctx.enter_context(tc.tile_pool(name="pp", bufs=6))
        outp = ctx.enter_context(tc.tile_pool(name="outp", bufs=4))

        XA = const.tile([128, PS], F16)
        XB = const.tile([128, PS], F16)
        XC = const.tile([C, PS], F16)
        WT = const.tile([128, 5 * 128], F16)
        BIAS = const.tile([128, 5], F32)
        IDT = const.tile([128, C], F16)

        # banded loads of the host-staged planes; the first chunk's needs
        # issue first so compute starts early.  WT/BIAS/IDT and the XC tail
        # go on the (otherwise idle) GPSIMD SWDGE queue so they load in
        # parallel with the Sync engine's band loads.
        # Three independent HWDGE queues carry the three staged planes so the
        # early bands land in parallel: XA on Sync, XB on Vector, XC on
        # Scalar.  The Vector/Scalar issue ops all enqueue at program start,
        # before those engines' first compute op, so they cost no steady-
        # state time.  WT leads the Sync queue; BIAS/IDT go via GPSIMD SWDGE.
        bands = [0, 6, 12, 24, 48, 80, H + 2]
        nc.sync.dma_start(WT[:], wt_in[:])
        nc.gpsimd.dma_start(BIAS[:], bias_in[:])
        nc.gpsimd.dma_start(IDT[:], idt_in[:])
        for b in range(len(bands) - 1):
            r0, r1 = bands[b], bands[b + 1]
            nc.scalar.dma_start(XC[:, r0 * W : r1 * W], xc_in[:, r0 * W : r1 * W])
            nc.sync.dma_start(XA[:, r0 * W : r1 * W], xa_in[:, r0 * W : r1 * W])
            nc.scalar.dma_start(XB[:, r0 * W : r1 * W], xb_in[:, r0 * W : r1 * W])

        def trio_window(off, count):
            """[128, count, Nc] sliding window over XA: consecutive windows
            start one row (W elems) apart; each spans CH rows contiguously."""
            base = XA[:, off : off + 1]
            w = base.copy()
            w.ap = mybir.VecI64Pair([tuple(w.ap[0]), (W, count), (1, Nc)])
            return w

        def fold(op_tile, n, src_ap, start, stop):
            """Fold matmul: contract the pair halves of one product tile into
            the op psum; even chunks land on psum partitions 0-63 (PE col
            group 0), odd chunks on 64-127 (col group 64)."""
            half = n % 2
            nc.tensor.matmul(
                op_tile[half * C : half * C + C, :],
                IDT[:],
                src_ap,
                start=start,
                stop=stop,
                tile_position=(0, half * C),
            )

        def evacuate(op_tile, g0, final=False):
            OUT = outp.tile([128, Nc], F16)
            nc.scalar.copy(OUT[:], op_tile[:])
            o3 = OUT[:].rearrange("p (a b) -> p a b", a=CH)
            # the last evacuation issues its stores on two queues in parallel
            eng = nc.scalar if final else nc.sync
            eng.dma_start(out_ext[:, g0 : g0 + CH, :], o3[0:C])
            nc.sync.dma_start(out_ext[:, g0 + CH : g0 + 2 * CH, :], o3[C:128])

        def emit_folds(op_tile, cn, srcs):
            for fi, src in enumerate(srcs):
                fold(
                    op_tile, cn, src,
                    start=(fi == 0), stop=(fi == len(srcs) - 1),
                )

        op = None  # op psum tile; covers chunks (2m, 2m+1)
        prev_folds = None  # chunk n-1's five product tiles (folded in chunk n)

        for n in range(nch):
            h0 = n * CH
            Ps = {}
            FB = fbp.tile([128, N_EXTRACT * Nc], F16, tag="fb")
            PT = ptp.tile([128, N_EXTRACT * Nc], F16, tag="pt")
            for jdx in EMIT_ORDER:
                ka, kb, tl, r = PAIRS[jdx]
                fp = fpsum.tile([128, Nc], F32, tag="fp")
                grp_b = jdx in GROUP_B
                r0 = h0 + 1
                if grp_b:
                    lhsT = WT[C:128, jdx * 128 : (jdx + 1) * 128]
                    rhs = XA[C:128, r0 * W : r0 * W + Nc]
                    tpos = (64, 0)
                else:
                    lhsT = WT[0:C, jdx * 128 : (jdx + 1) * 128]
                    rhs = XC[0:C, r0 * W : r0 * W + Nc]
                    tpos = (0, 0)
                nc.tensor.matmul(
                    fp[:], lhsT, rhs, start=True, stop=True, tile_position=tpos
                )
                if jdx < N_EXTRACT:
                    # ScalarE evacuates f (+bias) to SBUF fp16
                    nc.scalar.activation(
                        FB[:, jdx * Nc : (jdx + 1) * Nc],
                        fp[:],
                        Identity,
                        bias=BIAS[:, jdx : jdx + 1],
                    )
                else:
                    # DVE reads f straight from PSUM (1x) with bias fused
                    src = XA if tl == "A" else XB
                    in1 = src[:, (h0 + r) * W : (h0 + r) * W + Nc]
                    P = pp.tile([128, Nc], F16, tag="p")
                    nc.vector.scalar_tensor_tensor(
                        P[:], fp[:], BIAS[:, jdx : jdx + 1], in1, add, mult
                    )
                    Ps[jdx] = P[:]
            # one DVE op computes all three ScalarE-extracted pair products;
            # the last chunk uses three single-window ops instead so each
            # epilogue fold can start as soon as its own product is done
            FB3 = FB[:].rearrange("p (e n) -> p e n", e=N_EXTRACT)
            PT3 = PT[:].rearrange("p (e n) -> p e n", e=N_EXTRACT)
            if n < nch - 1:
                nc.vector.tensor_tensor(
                    PT3, FB3, trio_window(h0 * W, N_EXTRACT), mult
                )
            else:
                for e in range(N_EXTRACT):
                    nc.vector.tensor_tensor(
                        PT3[:, e : e + 1],
                        FB3[:, e : e + 1],
                        trio_window((h0 + e) * W, 1),
                        mult,
                    )

            # all of chunk n-1's folds run here: every product is a full
            # chunk old, so the PE never stalls on the DVE mid-chunk
            if prev_folds is not None:
                if n % 2 == 1:
                    op = opsum.tile([128, Nc], F32, tag="op")
                emit_folds(op, n - 1, prev_folds)
                if n % 2 == 0:
                    evacuate(op, (n - 2) * CH)
            prev_folds = [Ps[3], Ps[4]] + [
                PT[:, e * Nc : (e + 1) * Nc] for e in range(N_EXTRACT)
            ]

        # epilogue: last chunk's folds + final op evacuation
        emit_folds(op, nch - 1, prev_folds)
        evacuate(op, (nch - 2) * CH, final=True)

    nc.compile()
    return nc


_NC_CACHE = {}


def _get_nc(H, W, CH):
    key = (H, W, CH)
    if key not in _NC_CACHE:
        _NC_CACHE[key] = build_nc(H, W, CH)
    return _NC_CACHE[key]


def run(x, w_gen, b_gen, trace=False, tmpdir=None):
    x = np.asarray(x, dtype=np.float32)
    w_gen = np.asarray(w_gen, dtype=np.float32)
    b_gen = np.asarray(b_gen, dtype=np.float32)
    B, c, H, W = x.shape
    assert c == C and B == N_CORES

    wt, bias, idt = pack_weights(w_gen, b_gen)
    nc = _get_nc(H, W, 4)

    in_maps = []
    for i in range(B):
        XA, XB, XC = stage_planes(x[i], H, W)
        in_maps.append(
            {"xa": XA, "xb": XB, "xc": XC, "wt": wt, "bias": bias, "idt": idt}
        )
    res = run_bass_kernel_spmd(
        nc, in_maps, core_ids=list(range(N_CORES)), trace=trace, tmpdir=tmpdir
    )
    out = np.stack(
        [res.results[i]["out"].astype(np.float32) for i in range(B)], axis=0
    )
    return out, res


def kernel(x: np.ndarray, w_gen: np.ndarray, b_gen: np.ndarray) -> np.ndarray:
    return run(x, w_gen, b_gen)[0]


# revision 9
# speedup vs baseline: 1.0865x; 1.0401x over previous
import sys
from contextlib import ExitStack

import numpy as np

sys.path.insert(0, "/opt/trn_rl_repo")

import concourse.bass as bass  # noqa: E402
import concourse.mybir as mybir  # noqa: E402
import concourse.tile as tile  # noqa: E402
from concourse import bacc  # noqa: E402
from concourse.bass_utils import run_bass_kernel_spmd  # noqa: E402

C = 64
N_CORES = 8

# Pair table: each entry computes taps (ka, kb) of the 3x3 dynamic filter in
# one [K=64, M=128] matmul (top 64 psum partitions = ka's channels, bottom 64
# = kb's channels).  `tl` selects which staged plane-stack supplies the
# shifted patch operand; `r` is the row offset of the window within the
# padded 128-wide row layout.  Tap k=5 appears in both B-pairs with halved
# weights/bias so the products sum to the correct single contribution.
#   XA stack: top 64 partitions = padded x at col-offset 0 (j=0), bottom 64
#             = col-offset 1 (j=1).  Pair (3e, 3e+1) reads rows h0+e.
#   XB stack: top = col-offset 2 (j=2), bottom = j=2 shifted down one row.
PAIRS = [
    (0, 1, "A", 0),
    (3, 4, "A", 1),
    (6, 7, "A", 2),
    (2, 5, "B", 0),
    (5, 8, "B", 1),
]

F16 = mybir.dt.float16
F32 = mybir.dt.float32


def pack_weights(w_gen: np.ndarray, b_gen: np.ndarray):
    """Host-side packing of the tiny 1x1 generator weights.

    Returns
      wt   [128, 5*128] fp16 : lhsT blocks; block j col (half*64+c) holds
                               W[c, k, :] (tap k of pair j, half-scaled for
                               k=5); rows duplicated so PE row-group B
                               (partitions 64-127) can read its stationary
                               from the matching partition range
      bias [128, 5]    fp32 : per-partition bias for pair j
      idt  [128, 64]   fp16 : stacked identity [I64; I64] for the fold matmul
    """
    W = w_gen.reshape(C, 9, C).astype(np.float32)  # [c, k, c']
    b = b_gen.reshape(C, 9).astype(np.float32)
    wt = np.zeros((C, 5 * 128), np.float32)
    bias = np.zeros((128, 5), np.float32)
    for jdx, (ka, kb, _, _) in enumerate(PAIRS):
        for half, k in ((0, ka), (1, kb)):
            s = 0.5 if k == 5 else 1.0
            wt[:, jdx * 128 + half * 64 : jdx * 128 + half * 64 + C] = W[:, k, :].T * s
            bias[half * 64 : half * 64 + C, jdx] = b[:, k] * s
    idt = np.concatenate([np.eye(C), np.eye(C)], axis=0)
    wt2 = np.concatenate([wt, wt], axis=0)
    return wt2.astype(np.float16), bias.astype(np.float32), idt.astype(np.float16)


def stage_planes(x_core: np.ndarray, H: int, W: int):
    """Host-side staging: three 128-wide-row fp16 plane stacks whose window
    reads are all contiguous in the free dimension.

    With P = zero-padded x ([C, H+2, W+2]) and J_j[c, a, u] = P[c, a, u+j]:
      XA [128, (H+2)*W]: top 64 = J0, bottom 64 = J1
      XB [128, (H+2)*W]: top 64 = J2, bottom 64 = J2 shifted down one row
      XC [64,  (H+2)*W]: J1 (f-matmul rhs for PE row group A)
    """
    P = np.zeros((C, H + 2, W + 2), np.float16)
    P[:, 1 : H + 1, 1 : W + 1] = x_core
    J0 = P[:, :, 0:W].reshape(C, -1)
    J1 = P[:, :, 1 : W + 1].reshape(C, -1)
    J2 = P[:, :, 2 : W + 2].reshape(C, -1)
    J2dn = np.zeros_like(J2)
    J2dn[:, : -W] = J2[:, W:]  # row a -> row a+1
    XA = np.concatenate([J0, J1], axis=0)
    XB = np.concatenate([J2, J2dn], axis=0)
    return (
        np.ascontiguousarray(XA),
        np.ascontiguousarray(XB),
        np.ascontiguousarray(J1),
    )


def build_nc(H=128, W=128, CH=4):
    """Build the single-core Bass program (SPMD across cores).

    H, W: spatial dims; CH: image rows per chunk.
    """
    PS = (H + 2) * W  # plane size (rows of width W)
    Nc = CH * W  # pixels per chunk
    nch = H // CH
    assert Nc <= 512  # one psum bank / one matmul

    nc = bacc.Bacc("TRN2", target_bir_lowering=False)
    xa_in = nc.declare_dram_parameter("xa", [128, PS], F16, isOutput=False)
    xb_in = nc.declare_dram_parameter("xb", [128, PS], F16, isOutput=False)
    xc_in = nc.declare_dram_parameter("xc", [C, PS], F16, isOutput=False)
    wt_in = nc.declare_dram_parameter("wt", [128, 5 * 128], F16, isOutput=False)
    bias_in = nc.declare_dram_parameter("bias", [128, 5], F32, isOutput=False)
    idt_in = nc.declare_dram_parameter("idt", [128, C], F16, isOutput=False)
    out_ext = nc.declare_dram_parameter("out", [C, H, W], F16, isOutput=True)

    add = mybir.AluOpType.add
    mult = mybir.AluOpType.mult
    Identity = mybir.ActivationFunctionType.Identity

    N_EXTRACT = 3  # f-tiles extracted by ScalarE (bias fused there)
    GROUP_B = (3, 4)  # pairs streaming through PE rows 64-127
    EMIT_ORDER = [0, 3, 1, 4, 2]  # alternate A/B row-groups on the PE

    with ExitStack() as ctx:
        tc = ctx.enter_context(tile.TileContext(nc))
        const = ctx.enter_context(tc.tile_pool(name="const", bufs=1))
        fpsum = ctx.enter_context(tc.tile_pool(name="fpsum", bufs=6, space="PSUM"))
        opsum = ctx.enter_context(tc.tile_pool(name="opsum", bufs=2, space="PSUM"))
        fbp = ctx.enter_context(tc.tile_pool(name="fbp", bufs=3))
        ptp = ctx.enter_context(tc.tile_pool(name="ptp", bufs=4))
        pp = ctx.enter_context(tc.tile_pool(name="pp", bufs=6))
        outp = ctx.enter_context(tc.tile_pool(name="outp", bufs=4))

        XA = const.tile([128, PS], F16)
        XB = const.tile([128, PS], F16)
        XC = const.tile([C, PS], F16)
        WT = const.tile([128, 5 * 128], F16)
        BIAS = const.tile([128, 5], F32)
        IDT = const.tile([128, C], F16)

        # banded loads of the host-staged planes; the first chunk's needs
        # issue first so compute starts early.  WT/BIAS/IDT and the XC tail
        # go on the (otherwise idle) GPSIMD SWDGE queue so they load in
        # parallel with the Sync engine's band loads.
        # Three independent HWDGE queues carry the three staged planes so the
        # early bands land in parallel: XA on Sync, XB on Vector, XC on
        # Scalar.  The Vector/Scalar issue ops all enqueue at program start,
        # before those engines' first compute op, so they cost no steady-
        # state time.  WT leads the Sync queue; BIAS/IDT go via GPSIMD SWDGE.
        # XA+XB stream on the Sync HWDGE queue (fine early bands for ramp
        # latency); XC rides the Scalar queue with only 4 issue ops, all
        # enqueued before ScalarE's first extract (deeper queues block the
        # issuing engine — HWDGE depth is ~5).  BIAS/IDT go via GPSIMD SWDGE.
        bands = [0, 6, 12, 24, 48, 80, H + 2]
        xc_bands = [0, 6, 24, 64, H + 2]
        nc.sync.dma_start(WT[:], wt_in[:])
        nc.gpsimd.dma_start(BIAS[:], bias_in[:])
        nc.gpsimd.dma_start(IDT[:], idt_in[:])
        for b in range(len(xc_bands) - 1):
            r0, r1 = xc_bands[b], xc_bands[b + 1]
            nc.scalar.dma_start(XC[:, r0 * W : r1 * W], xc_in[:, r0 * W : r1 * W])
        for b in range(len(bands) - 1):
            r0, r1 = bands[b], bands[b + 1]
            nc.sync.dma_start(XA[:, r0 * W : r1 * W], xa_in[:, r0 * W : r1 * W])
            nc.sync.dma_start(XB[:, r0 * W : r1 * W], xb_in[:, r0 * W : r1 * W])

        def trio_window(off, count):
            """[128, count, Nc] sliding window over XA: consecutive windows
            start one row (W elems) apart; each spans CH rows contiguously."""
            base = XA[:, off : off + 1]
            w = base.copy()
            w.ap = mybir.VecI64Pair([tuple(w.ap[0]), (W, count), (1, Nc)])
            return w

        def fold(op_tile, n, src_ap, start, stop):
            """Fold matmul: contract the pair halves of one product tile into
            the op psum; even chunks land on psum partitions 0-63 (PE col
            group 0), odd chunks on 64-127 (col group 64)."""
            half = n % 2
            nc.tensor.matmul(
                op_tile[half * C : half * C + C, :],
                IDT[:],
                src_ap,
                start=start,
                stop=stop,
                tile_position=(0, half * C),
            )

        def evacuate(op_tile, g0, final=False):
            OUT = outp.tile([128, Nc], F16)
            nc.scalar.copy(OUT[:], op_tile[:])
            o3 = OUT[:].rearrange("p (a b) -> p a b", a=CH)
            # the last evacuation issues its stores on two queues in parallel
            eng = nc.scalar if final else nc.sync
            eng.dma_start(out_ext[:, g0 : g0 + CH, :], o3[0:C])
            nc.sync.dma_start(out_ext[:, g0 + CH : g0 + 2 * CH, :], o3[C:128])

        def emit_folds(op_tile, cn, srcs):
            for fi, src in enumerate(srcs):
                fold(
                    op_tile, cn, src,
                    start=(fi == 0), stop=(fi == len(srcs) - 1),
                )

        op = None  # op psum tile; covers chunks (2m, 2m+1)
        prev_folds = None  # chunk n-1's five product tiles (folded in chunk n)

        for n in range(nch):
            h0 = n * CH
            Ps = {}
            FB = fbp.tile([128, N_EXTRACT * Nc], F16, tag="fb")
            PT = ptp.tile([128, N_EXTRACT * Nc], F16, tag="pt")
            for jdx in EMIT_ORDER:
                ka, kb, tl, r = PAIRS[jdx]
                fp = fpsum.tile([128, Nc], F32, tag="fp")
                grp_b = jdx in GROUP_B
                r0 = h0 + 1
                if grp_b:
                    lhsT = WT[C:128, jdx * 128 : (jdx + 1) * 128]
                    rhs = XA[C:128, r0 * W : r0 * W + Nc]
                    tpos = (64, 0)
                else:
                    lhsT = WT[0:C, jdx * 128 : (jdx + 1) * 128]
                    rhs = XC[0:C, r0 * W : r0 * W + Nc]
                    tpos = (0, 0)
                nc.tensor.matmul(
                    fp[:], lhsT, rhs, start=True, stop=True, tile_position=tpos
                )
                if jdx < N_EXTRACT:
                    # ScalarE evacuates f (+bias) to SBUF fp16
                    nc.scalar.activation(
                        FB[:, jdx * Nc : (jdx + 1) * Nc],
                        fp[:],
                        Identity,
                        bias=BIAS[:, jdx : jdx + 1],
                    )
                else:
                    # DVE reads f straight from PSUM (1x) with bias fused
                    src = XA if tl == "A" else XB
                    in1 = src[:, (h0 + r) * W : (h0 + r) * W + Nc]
                    P = pp.tile([128, Nc], F16, tag="p")
                    nc.vector.scalar_tensor_tensor(
                        P[:], fp[:], BIAS[:, jdx : jdx + 1], in1, add, mult
                    )
                    Ps[jdx] = P[:]
            # one DVE op computes all three ScalarE-extracted pair products;
            # the last chunk uses three single-window ops instead so each
            # epilogue fold can start as soon as its own product is done
            FB3 = FB[:].rearrange("p (e n) -> p e n", e=N_EXTRACT)
            PT3 = PT[:].rearrange("p (e n) -> p e n", e=N_EXTRACT)
            if n < nch - 1:
                nc.vector.tensor_tensor(
                    PT3, FB3, trio_window(h0 * W, N_EXTRACT), mult
                )
            else:
                for e in range(N_EXTRACT):
                    nc.vector.tensor_tensor(
                        PT3[:, e : e + 1],
                        FB3[:, e : e + 1],
                        trio_window((h0 + e) * W, 1),
                        mult,
                    )

            # all of chunk n-1's folds run here: every product is a full
            # chunk old, so the PE never stalls on the DVE mid-chunk
            if prev_folds is not None:
                if n % 2 == 1:
                    op = opsum.tile([128, Nc], F32, tag="op")
                emit_folds(op, n - 1, prev_folds)
                if n % 2 == 0:
                    evacuate(op, (n - 2) * CH)
            prev_folds = [Ps[3], Ps[4]] + [
                PT[:, e * Nc : (e + 1) * Nc] for e in range(N_EXTRACT)
            ]

        # epilogue: last chunk's folds + final op evacuation
        emit_folds(op, nch - 1, prev_folds)
        evacuate(op, (nch - 2) * CH, final=True)

    nc.compile()
    return nc


_NC_CACHE = {}


def _get_nc(H, W, CH):
    key = (H, W, CH)
    if key not in _NC_CACHE:
        _NC_CACHE[key] = build_nc(H, W, CH)
    return _NC_CACHE[key]


def run(x, w_gen, b_gen, trace=False, tmpdir=None):
    x = np.asarray(x, dtype=np.float32)
    w_gen = np.asarray(w_gen, dtype=np.float32)
    b_gen = np.asarray(b_gen, dtype=np.float32)
    B, c, H, W = x.shape
    assert c == C and B == N_CORES

    wt, bias, idt = pack_weights(w_gen, b_gen)
    nc = _get_nc(H, W, 4)

    in_maps = []
    for i in range(B):
        XA, XB, XC = stage_planes(x[i], H, W)
        in_maps.append(
            {"xa": XA, "xb": XB, "xc": XC, "wt": wt, "bias": bias, "idt": idt}
        )
    res = run_bass_kernel_spmd(
        nc, in_maps, core_ids=list(range(N_CORES)), trace=trace, tmpdir=tmpdir
    )
    out = np.stack(
        [res.results[i]["out"].astype(np.float32) for i in range(B)], axis=0
    )
    return out, res


def kernel(x: np.ndarray, w_gen: np.ndarray, b_gen: np.ndarray) -> np.ndarray:
    return run(x, w_gen, b_gen)[0]
